# revision 1
# baseline (speedup 1.0000x reference)
"""Trainium2 Bass kernel for nn_BilinearAttention2 (gnn_message_passing).

Math (per graph g, head h — where "head" h is a raw C-order reshape of the
[nA, D] block into [H, nA, HD], i.e. head h = 16 consecutive nodes reshaped):
  x1 = A @ W1.T + b1 ; x2 = B @ W2.T + b2
  X1 = x1[g].flat[h*4096:(h+1)*4096].reshape(128, 32)   (likewise X2)
  att[i,j]  = sum_k tanh(X1[i,k] * X2[j,k]) * q[k]
  b2a = softmax_i(mean_j att); a2b = softmax_j(mean_i att)
  A_p[g,h] = X1.T @ b2a ; B_p[g,h] = X2.T @ a2b
  out[g] = concat(A_p[g].flat, B_p[g].flat)    -> [G, 2D]

Sharding: data-parallel over graphs. 8 cores x 2 graphs each; weights
replicated. Each core processes its 16 (g,h) pairs as 4 "stacks" of 4 pairs:
partition dim = (pair-in-stack, k) = (4, 32) = 128, free dim = (i', j') = 16384.
One DVE broadcast-multiply + one ACT tanh per stack; the q-weighted k-sum is a
PE matmul with a block-diagonal q lhsT; both softmax reductions then act on the
small per-pair S matrices.
"""
import sys

sys.path.insert(0, "/opt/trn_rl_repo")

from contextlib import ExitStack

import numpy as np

import concourse.bass as bass
import concourse.bacc as bacc
import concourse.mybir as mybir
import concourse.tile as tile
from concourse.masks import make_identity

F32 = mybir.dt.float32
BF16 = mybir.dt.bfloat16

D = 256
H = 8
HD = 32
G = 16
NA = 128
NB = 128
NCORES = 8
GSH = G // NCORES          # graphs per core = 2
NPAIR = GSH * H            # 16 (g,h) pairs per core
SPP = 4                    # pairs per stack
NSTACK = NPAIR // SPP      # 4
NK = HD                    # 32
NJ = 128                   # nodes per head-view
FF = NJ * NJ               # 16384 free elems per stack


def build_kernel():
    nc = bacc.Bacc()
    a_d = nc.dram_tensor("A", [GSH * NA, D], F32, kind="ExternalInput")
    b_d = nc.dram_tensor("B", [GSH * NB, D], F32, kind="ExternalInput")
    w1_d = nc.dram_tensor("W1", [D, D], F32, kind="ExternalInput")
    w2_d = nc.dram_tensor("W2", [D, D], F32, kind="ExternalInput")
    b1_d = nc.dram_tensor("bias1", [1, D], F32, kind="ExternalInput")
    b2_d = nc.dram_tensor("bias2", [1, D], F32, kind="ExternalInput")
    q_d = nc.dram_tensor("q", [1, NK], F32, kind="ExternalInput")
    out_d = nc.dram_tensor("out", [GSH, 2 * D], F32, kind="ExternalOutput")

    # DRAM scratch
    x1_dram = nc.dram_tensor("x1_scratch", [GSH * NA, D], F32)
    x2_dram = nc.dram_tensor("x2_scratch", [GSH * NB, D], F32)

    with tile.TileContext(nc) as tc, ExitStack() as ctx:
        cst = ctx.enter_context(tc.tile_pool(name="cst", bufs=1))
        sbin = ctx.enter_context(tc.tile_pool(name="sbin", bufs=1))
        sbt = ctx.enter_context(tc.tile_pool(name="sbt", bufs=1))
        sbtr = ctx.enter_context(tc.tile_pool(name="sbtr", bufs=2))
        big = ctx.enter_context(tc.tile_pool(name="big", bufs=2))
        fold1 = ctx.enter_context(tc.tile_pool(name="fold1", bufs=1))
        sm = ctx.enter_context(tc.tile_pool(name="sm", bufs=2))
        pst = ctx.enter_context(tc.tile_pool(name="pst", bufs=2, space="PSUM"))
        psx = ctx.enter_context(tc.tile_pool(name="psx", bufs=2, space="PSUM"))
        pss = ctx.enter_context(tc.tile_pool(name="pss", bufs=2, space="PSUM"))
        ps1 = ctx.enter_context(tc.tile_pool(name="ps1", bufs=1, space="PSUM"))

        ident = cst.tile([128, 128], F32)
        make_identity(nc, ident[:])
        ones1 = cst.tile([1, 128], F32)
        nc.vector.memset(ones1[:], 1.0)
        onescol = cst.tile([128, 1], F32)
        nc.vector.memset(onescol[:], 1.0)

        # ---- load inputs; transpose W1,W2 fully, A/B per row-block ----
        _ldq = [nc.sync, nc.scalar, nc.gpsimd]

        def trans_rowblock(src_d, t_sb, r, name, qi=[0]):
            """transpose rows [128r, 128r+128) of src_d into t_sb[c][:, 128r:+128]"""
            blk = sbin.tile([128, D], F32, tag=f"{name}ld")
            _ldq[qi[0] % 3].dma_start(blk[:], src_d[r * 128:(r + 1) * 128, :])
            qi[0] += 1
            for c in range(2):
                tp = pst.tile([128, 128], F32, tag="tr")
                nc.tensor.transpose(tp[:], blk[:, c * 128:(c + 1) * 128], ident[:])
                nc.vector.tensor_copy(t_sb[c][:, r * 128:(r + 1) * 128], tp[:])

        def alloc_t(name):
            return [sbin.tile([128, D], F32, tag=f"{name}T{c}", name=f"{name}T{c}") for c in range(2)]

        at, bt, w1t, w2t = alloc_t("A"), alloc_t("B"), alloc_t("W1"), alloc_t("W2")
        b1_sb = sbin.tile([1, D], F32)
        b2_sb = sbin.tile([1, D], F32)
        # g0-critical loads first, spread across queues
        trans_rowblock(b_d, bt, 0, "B")
        trans_rowblock(w2_d, w2t, 0, "W2")
        trans_rowblock(w2_d, w2t, 1, "W2")
        trans_rowblock(a_d, at, 0, "A")
        trans_rowblock(w1_d, w1t, 0, "W1")
        trans_rowblock(w1_d, w1t, 1, "W1")
        nc.sync.dma_start(b2_sb[:], b2_d[:])
        nc.scalar.dma_start(b1_sb[:], b1_d[:])

        def xmm(xt, wt, bb, xd, g):
            xp = psx.tile([128, D], F32, tag="xmm")
            nc.tensor.matmul(xp[:], xt[0][:, g * 128:(g + 1) * 128], wt[0][:], start=True, stop=False)
            nc.tensor.matmul(xp[:], xt[1][:, g * 128:(g + 1) * 128], wt[1][:], start=False, stop=False)
            nc.tensor.matmul(xp[:], ones1[0:1, :], bb[:], start=False, stop=True)
            xs = sbin.tile([128, D], F32, tag="xsb")
            nc.vector.tensor_copy(xs[:], xp[:])
            nc.sync.dma_start(xd[g * 128:(g + 1) * 128, :], xs[:])

        x1f, x2f, x1t, x2t = [None] * NSTACK, [None] * NSTACK, [None] * NSTACK, [None] * NSTACK

        def gather_stack(s):
            g, hq = s // 2, s % 2
            for (xd, fl, tl, nm, quad) in ((x1_dram, x1f, x1t, "x1", True), (x2_dram, x2f, x2t, "x2", False)):
                xf = sbt.tile([128, 128], F32, tag=f"{nm}f{s}")
                srcv = xd[:].rearrange("(g2 hq pp n) (ss k) -> g2 hq n ss pp k", g2=GSH, hq=2, pp=SPP, ss=H)
                nc.sync.dma_start(xf[:], srcv[g, hq])
                fl[s] = xf
                tp = pst.tile([128, 128], F32, tag="tr")
                nc.tensor.transpose(tp[:], xf[:], ident[:])
                if quad:
                    xtb = sbtr.tile([128, 4 * 128], BF16, tag=f"{nm}tq")
                    nc.vector.tensor_copy(xtb[:].rearrange("p (i q) -> p i q", q=4),
                                          tp[:].unsqueeze(2).broadcast_to([128, 128, 4]))
                else:
                    xtb = sbtr.tile([128, 128], BF16, tag=f"{nm}tp")
                    nc.vector.tensor_copy(xtb[:], tp[:])
                tl[s] = xtb

        # ---- qdiag [128, SPP]: qdiag[(pp,k), pp'] = q[k] * (pp == pp') ----
        q_sb = cst.tile([1, NK], F32)
        nc.scalar.dma_start(q_sb[:], q_d[:])
        q_bf = cst.tile([1, NK], BF16)
        nc.vector.tensor_copy(q_bf[:], q_sb[:])
        qdiag = cst.tile([128, SPP], BF16)
        nc.vector.memset(qdiag[:], 0.0)
        for pp in range(SPP):
            nc.scalar.dma_start(qdiag[pp * NK:(pp + 1) * NK, pp:pp + 1], q_bf[:])

        # graph-0 chain first so stack 0 starts ASAP (x2 leads: it trails otherwise)
        xmm(bt, w2t, b2_sb, x2_dram, 0)
        xmm(at, w1t, b1_sb, x1_dram, 0)
        gather_stack(0)
        gather_stack(1)
        trans_rowblock(a_d, at, 1, "A")
        trans_rowblock(b_d, bt, 1, "B")
        xmm(bt, w2t, b2_sb, x2_dram, 1)
        xmm(at, w1t, b1_sb, x1_dram, 1)
        gather_stack(2)
        gather_stack(3)

        # ---- main loop over stacks (compute + per-stack epilogue) ----
        probs_at = sm.tile([128, NPAIR], F32, tag="pta")
        probs_bt = sm.tile([128, NPAIR], F32, tag="ptb")
        for s in range(NSTACK):
            g, hq = s // 2, s % 2
            p4 = big.tile([128, FF], BF16, tag="p4")
            t4 = big.tile([128, FF], BF16, tag="t4")
            if s == 0:
                widths_i = (16, 16, 32, 64)
            elif s == NSTACK - 1:
                widths_i = (32, 32, 32, 32)
            else:
                widths_i = (64, 64)
            io = 0
            for HH in widths_i:
                in0 = x1t[s][:, io * 4:(io + HH) * 4].rearrange("p (i q) -> p i q", q=4)\
                    .unsqueeze(2).broadcast_to([128, HH, NJ // 4, 4])
                in1 = x2t[s][:].rearrange("p (j2 q) -> p j2 q", q=4)\
                    .unsqueeze(1).broadcast_to([128, HH, NJ // 4, 4])
                nc.vector.tensor_tensor(
                    p4[:, io * NJ:(io + HH) * NJ].rearrange("p (i j2 q) -> p i j2 q", q=4, j2=NJ // 4),
                    in0, in1, op=mybir.AluOpType.mult)
                nc.scalar.activation(t4[:, io * NJ:(io + HH) * NJ], p4[:, io * NJ:(io + HH) * NJ],
                                     mybir.ActivationFunctionType.Tanh)
                io += HH

            # --- a2b: accumulate i'-quads on PE with q-block-diag lhsT ---
            a2b_ps = pss.tile([SPP, 4 * NJ], F32, tag="a2b")
            nch = FF // (4 * NJ)
            for ch in range(nch):
                nc.tensor.matmul(a2b_ps[:], qdiag[:], t4[:, ch * 4 * NJ:(ch + 1) * 4 * NJ],
                                 start=(ch == 0), stop=(ch == nch - 1))
            a2b_t = sm.tile([SPP, NJ], F32, tag="a2bt")
            nc.vector.tensor_reduce(
                a2b_t[:], a2b_ps[:].rearrange("p (iq j) -> p j iq", iq=4),
                axis=mybir.AxisListType.X, op=mybir.AluOpType.add)

            # --- b2a: log-fold T4 over j' on GPS, then qdiag mm ---
            m1 = fold1.tile([128, FF // 2], BF16, tag="m1")
            msc = fold1.tile([128, 7936], F32, tag="msc")
            cj4 = sm.tile([128, NJ], BF16, tag="cj4")
            nq = 4 if s == NSTACK - 1 else 2
            QW = FF // nq          # t4 elems per i'-chunk
            for qt in range(nq):
                feng = nc.vector if (s == NSTACK - 1 and qt == 2) else nc.gpsimd
                vh = t4[:, qt * QW:(qt + 1) * QW].rearrange("p (ij2 two) -> p ij2 two", two=2)
                feng.tensor_tensor(m1[:, qt * QW // 2:(qt + 1) * QW // 2],
                                   vh[:, :, 0:1].squeeze(2), vh[:, :, 1:2].squeeze(2),
                                   op=mybir.AluOpType.add)
                prev_ap = m1[:, qt * QW // 2:(qt + 1) * QW // 2]
                widths = []
                ww = QW // 4
                while ww >= 2 * NJ // nq:
                    widths.append(ww)
                    ww //= 2
                base = 0
                for wdt in widths:
                    seg = msc[:, base + qt * wdt: base + (qt + 1) * wdt]
                    pv = prev_ap.rearrange("p (ij2 two) -> p ij2 two", two=2)
                    feng.tensor_tensor(seg, pv[:, :, 0:1].squeeze(2), pv[:, :, 1:2].squeeze(2),
                                       op=mybir.AluOpType.add)
                    prev_ap = seg
                    base += wdt * nq
                pv = prev_ap.rearrange("p (ij2 two) -> p ij2 two", two=2)
                feng.tensor_tensor(cj4[:, qt * NJ // nq:(qt + 1) * NJ // nq],
                                   pv[:, :, 0:1].squeeze(2), pv[:, :, 1:2].squeeze(2),
                                   op=mybir.AluOpType.add)
            b2a_ps = ps1.tile([SPP, NJ], F32, tag="b2a")
            QN = NJ // nq
            for qt in range(nq):
                nc.tensor.matmul(b2a_ps[:, qt * QN:(qt + 1) * QN], qdiag[:],
                                 cj4[:, qt * QN:(qt + 1) * QN], start=True, stop=True)

            # --- per-stack softmax + probsT (prob vectors live in cols 4s..4s+4) ---
            for (lg, pt, nm) in ((b2a_ps, probs_at, "a"), (a2b_t, probs_bt, "b")):
                ex = sm.tile([SPP, NJ], F32, tag=f"ex{nm}")
                nc.scalar.activation(ex[:], lg[:], mybir.ActivationFunctionType.Exp, scale=1.0 / NJ)
                sme = sm.tile([SPP, 1], F32, tag=f"sm{nm}")
                nc.vector.tensor_reduce(sme[:], ex[:], axis=mybir.AxisListType.X, op=mybir.AluOpType.add)
                rcp = sm.tile([SPP, 1], F32, tag=f"rc{nm}")
                nc.vector.reciprocal(rcp[:], sme[:])
                pr = sm.tile([SPP, NJ], F32, tag=f"pr{nm}")
                nc.vector.tensor_scalar_mul(pr[:], ex[:], rcp[:])
                pp_ps = pst.tile([128, SPP], F32, tag="tr")
                nc.tensor.transpose(pp_ps[:], pr[:], ident[0:SPP, 0:SPP])
                nc.vector.tensor_copy(pt[:, s * SPP:(s + 1) * SPP], pp_ps[:])

            # --- projections + output ---
            for (xf, pt, half) in ((x1f[s], probs_at, 0), (x2f[s], probs_bt, 1)):
                pj = ps1.tile([NK, SPP], F32, tag="proj")
                for pp in range(SPP):
                    nc.tensor.matmul(pj[:, pp:pp + 1], xf[:, pp * NK:(pp + 1) * NK],
                                     pt[:, s * SPP + pp:s * SPP + pp + 1], start=True, stop=True)
                pjs = sm.tile([NK, SPP], F32, tag=f"projs{s}_{half}")
                nc.vector.tensor_copy(pjs[:], pj[:])
                dst = out_d[g:g + 1, half * D + hq * SPP * HD: half * D + (hq + 1) * SPP * HD]
                dst = dst.rearrange("o (pp k) -> o k pp", pp=SPP)
                nc.sync.dma_start(dst, pjs[:])

    if not nc.is_finalized():
        nc.finalize()
    return nc


def shard_inputs(inputs):
    """Full inputs -> list of 8 per-core input maps."""
    A = np.asarray(inputs["A"], np.float32)
    B = np.asarray(inputs["B"], np.float32)
    maps = []
    for c in range(NCORES):
        maps.append({
            "A": np.ascontiguousarray(A[c * GSH * NA:(c + 1) * GSH * NA]),
            "B": np.ascontiguousarray(B[c * GSH * NB:(c + 1) * GSH * NB]),
            "W1": np.asarray(inputs["W1"], np.float32),
            "W2": np.asarray(inputs["W2"], np.float32),
            "bias1": np.asarray(inputs["bias1"], np.float32).reshape(1, D),
            "bias2": np.asarray(inputs["bias2"], np.float32).reshape(1, D),
            "q": np.asarray(inputs["q"], np.float32).reshape(1, NK),
        })
    return maps


_NC_CACHE = {}


def kernel(**inputs) -> np.ndarray:
    """Full (unsharded) inputs -> full [G, 2D] output, running on 8 cores."""
    from concourse.bass_utils import run_bass_kernel_spmd

    if "nc" not in _NC_CACHE:
        _NC_CACHE["nc"] = build_kernel()
    nc = _NC_CACHE["nc"]
    in_maps = shard_inputs(inputs)
    res = run_bass_kernel_spmd(nc, in_maps, core_ids=list(range(NCORES)))
    out = np.concatenate([res.results[c]["out"] for c in range(NCORES)], axis=0)
    return out.astype(np.float32)


if __name__ == "__main__":
    # CoreSim single-core debug: core 0 vs numpy reference
    from concourse.bass_interp import CoreSim

    rng = np.random.default_rng(0)
    scale = 1.0 / np.sqrt(D)
    full = {
        "A": rng.standard_normal((G * NA, D)).astype(np.float32),
        "B": rng.standard_normal((G * NB, D)).astype(np.float32),
        "W1": (rng.standard_normal((D, D)) * scale).astype(np.float32),
        "bias1": (rng.standard_normal(D) * scale).astype(np.float32),
        "W2": (rng.standard_normal((D, D)) * scale).astype(np.float32),
        "bias2": (rng.standard_normal(D) * scale).astype(np.float32),
        "q": (rng.standard_normal(HD) * scale).astype(np.float32),
    }

    def ref_core(m):
        x1 = m["A"] @ m["W1"].T + m["bias1"][0]
        x2 = m["B"] @ m["W2"].T + m["bias2"][0]
        x1 = x1.reshape(GSH, H, NA, HD)
        x2 = x2.reshape(GSH, H, NB, HD)
        att = np.einsum("ghijk,k->ghij", np.tanh(x1[:, :, :, None, :] * x2[:, :, None, :, :]), m["q"][0])

        def smax(v, ax):
            v = v - v.max(axis=ax, keepdims=True)
            e = np.exp(v)
            return e / e.sum(axis=ax, keepdims=True)

        b2a = smax(att.mean(axis=3), 2)
        a2b = smax(att.mean(axis=2), 2)
        A_p = np.einsum("ghik,ghi->ghk", x1, b2a).reshape(GSH, D)
        B_p = np.einsum("ghjk,ghj->ghk", x2, a2b).reshape(GSH, D)
        return np.concatenate([A_p, B_p], axis=1)

    nc = build_kernel()
    m0 = shard_inputs(full)[0]
    sim = CoreSim(nc)
    for k, v in m0.items():
        sim.tensor(k)[:] = v
    sim.simulate()
    got = sim.tensor("out").copy()
    want = ref_core(m0)
    err = np.abs(got - want).max() / np.abs(want).max()
    print("sim time:", sim.time, "ns")
    print("rel err:", err)



# revision 4
# speedup vs baseline: 1011.6111x; 1011.6111x over previous
"""Trainium2 Bass kernel v2 for nn_BilinearAttention2 (gnn_message_passing).

Math (per graph g, head h; head h = raw C-order view of x[g] [128,256] as
[8,128,32], i.e. 16 consecutive rows -> 128 pseudo-nodes x 32 dims):
  x1 = A @ W1.T + b1 ; x2 = B @ W2.T + b2
  att[i,j]  = sum_k tanh(X1[i,k] * X2[j,k]) * q[k]
  b2a = softmax_i(mean_j att); a2b = softmax_j(mean_i att)
  A_p[g,h] = X1.T @ b2a ; B_p[g,h] = X2.T @ a2b ; out[g] = [A_p | B_p]

Sharding: data-parallel over graphs: 8 cores x 2 graphs. Per core 16 (g,h)
pairs processed as 4 stacks of 4: partition = (pair, k) = (4,32)=128,
free = (i,j) = 16384.

Engine plan (empirical CoreSim costs, per [128,16384] stack):
  DVE : broadcast mult p4 = x1 (x) x2 (bf16 2x mode)      ~8.8us
  ACT : t4 = tanh(p4) -- the hard 14.4us/stack bottleneck (1 elem/cy/part)
  Pool: fold t4 over j: 128 -> 64 -> 32 -> 8              ~9.4us
  PE  : a2b = accumulating qdiag-matmuls over t4 512-col chunks
        (i-sum lands in PSUM accumulation); b2a = accumulating
        qdiag-matmuls over the j-folded t4 (2 per chunk)  ~12us
  DVE : [4,512] strided PSUM reduces, batched exp softmax, projections.
Program order interleaves graph-1 prologue inside stack 0's tanh window;
chunk widths taper (16,16,32,64 / ... / 64,32,16,16) to shrink pipeline
fill/drain. ACT issues no DMAs (dispatches cost ~500ns of engine time).

build_kernel(reps=N) wraps the whole per-call body in a hardware loop:
e2e(reps hi) - e2e(reps lo) isolates true per-iteration HW time from the
~68ms axon dispatch floor.
"""
import sys

sys.path.insert(0, "/opt/trn_rl_repo")

from contextlib import ExitStack

import numpy as np

import concourse.bass as bass
import concourse.bacc as bacc
import concourse.mybir as mybir
import concourse.tile as tile
from concourse.masks import make_identity

F32 = mybir.dt.float32
BF16 = mybir.dt.bfloat16

D = 256
H = 8
HD = 32
G = 16
NA = 128
NB = 128
NCORES = 8
GSH = G // NCORES          # graphs per core = 2
NPAIR = GSH * H            # 16 (g,h) pairs per core
SPP = 4                    # pairs per stack
NSTACK = NPAIR // SPP      # 4
NK = HD                    # 32
NJ = 128                   # nodes per head-view
FF = NJ * NJ               # 16384 free elems per stack

WIDTHS = [
    (16, 16, 32, 32, 32),   # stack 0: small first chunks -> ACT starts early
    (32, 32, 32, 32),
    (32, 32, 32, 32),
    (32, 32, 32, 24, 8),    # stack 3: small last chunk -> short drain tail
]


def build_kernel(reps: int = 1):
    nc = bacc.Bacc()
    a_d = nc.dram_tensor("A", [GSH * NA, D], F32, kind="ExternalInput")
    b_d = nc.dram_tensor("B", [GSH * NB, D], F32, kind="ExternalInput")
    w1_d = nc.dram_tensor("W1", [D, D], F32, kind="ExternalInput")
    w2_d = nc.dram_tensor("W2", [D, D], F32, kind="ExternalInput")
    b1_d = nc.dram_tensor("bias1", [1, D], F32, kind="ExternalInput")
    b2_d = nc.dram_tensor("bias2", [1, D], F32, kind="ExternalInput")
    q_d = nc.dram_tensor("q", [1, NK], F32, kind="ExternalInput")
    out_d = nc.dram_tensor("out", [GSH, 2 * D], F32, kind="ExternalOutput")

    # DRAM scratch for the head-view gather (bf16: half the traffic)
    x1_dram = nc.dram_tensor("x1_scratch", [GSH * NA, D], BF16)
    x2_dram = nc.dram_tensor("x2_scratch", [GSH * NB, D], BF16)

    with tile.TileContext(nc) as tc, ExitStack() as ctx:
        cst = ctx.enter_context(tc.tile_pool(name="cst", bufs=1))
        sbin = ctx.enter_context(tc.tile_pool(name="sbin", bufs=1))
        sbt = ctx.enter_context(tc.tile_pool(name="sbt", bufs=1))
        sbtr = ctx.enter_context(tc.tile_pool(name="sbtr", bufs=2))
        big = ctx.enter_context(tc.tile_pool(name="big", bufs=2))
        jfp = ctx.enter_context(tc.tile_pool(name="jfp", bufs=2))
        jf2p = ctx.enter_context(tc.tile_pool(name="jf2p", bufs=2))
        jf3p = ctx.enter_context(tc.tile_pool(name="jf3p", bufs=2))
        sm = ctx.enter_context(tc.tile_pool(name="sm", bufs=2))
        pst = ctx.enter_context(tc.tile_pool(name="pst", bufs=2, space="PSUM"))
        pstb = ctx.enter_context(tc.tile_pool(name="pstb", bufs=1, space="PSUM"))
        psx = ctx.enter_context(tc.tile_pool(name="psx", bufs=1, space="PSUM"))
        psacc = ctx.enter_context(tc.tile_pool(name="psacc", bufs=3, space="PSUM"))
        psproj = ctx.enter_context(tc.tile_pool(name="psproj", bufs=1, space="PSUM"))

        ident = cst.tile([128, 128], F32)
        make_identity(nc, ident[:])
        ident_bf = cst.tile([128, 128], BF16)
        nc.vector.tensor_copy(ident_bf[:], ident[:])
        ones1 = cst.tile([1, 128], BF16)
        nc.vector.memset(ones1[:], 1.0)

        def body():
            # ---------------- prologue: loads + weight transposes ----------------
            def load_block(src_d, r, name, eng):
                blk = sbin.tile([128, D], F32, tag=f"{name}ld")
                eng.dma_start(blk[:], src_d[r * 128:(r + 1) * 128, :])
                return blk

            def trans_block(blk, t_sb, r, use_act=False):
                # NB: Pool/GPSIMD cannot touch PSUM on real HW (BIR verifier),
                # so PSUM->SBUF copies go to DVE or, in the idle prologue, ACT.
                for c in range(2):
                    tp = pst.tile([128, 128], F32, tag="tr")
                    nc.tensor.transpose(tp[:], blk[:, c * 128:(c + 1) * 128], ident[:])
                    if use_act:
                        nc.scalar.copy(t_sb[c][:, r * 128:(r + 1) * 128], tp[:])
                    else:
                        nc.vector.tensor_copy(t_sb[c][:, r * 128:(r + 1) * 128], tp[:])

            def alloc_t(name):
                return [sbin.tile([128, D], BF16, tag=f"{name}T{c}", name=f"{name}T{c}")
                        for c in range(2)]

            at, bt, w1t, w2t = alloc_t("A"), alloc_t("B"), alloc_t("W1"), alloc_t("W2")
            b1_sb = sbin.tile([1, D], F32)
            b2_sb = sbin.tile([1, D], F32)
            b1_bf = sbin.tile([1, D], BF16, tag="b1bf")
            b2_bf = sbin.tile([1, D], BF16, tag="b2bf")
            q_sb = sbin.tile([1, NK], F32, tag="q")
            q_bf = sbin.tile([1, NK], BF16, tag="qbf")
            qdiag = sbin.tile([128, SPP], BF16, tag="qdiag")

            # parallel dispatch of g0-critical loads across non-ACT queues
            blk_w1_0 = load_block(w1_d, 0, "W1", nc.sync)
            blk_w1_1 = load_block(w1_d, 1, "W1b", nc.sync)
            blk_a_0 = load_block(a_d, 0, "A", nc.gpsimd)
            nc.scalar.dma_start(b1_sb[:], b1_d[:])
            nc.scalar.dma_start(b2_sb[:], b2_d[:])
            nc.scalar.dma_start(q_sb[:], q_d[:])
            blk_w2_0 = load_block(w2_d, 0, "W2", nc.gpsimd)
            blk_w2_1 = load_block(w2_d, 1, "W2b", nc.scalar)
            blk_b_0 = load_block(b_d, 0, "B", nc.scalar)
            nc.gpsimd.tensor_copy(b2_bf[:], b2_sb[:])
            nc.gpsimd.tensor_copy(b1_bf[:], b1_sb[:])
            nc.gpsimd.tensor_copy(q_bf[:], q_sb[:])
            nc.gpsimd.memset(qdiag[:], 0.0)

            def xmm(xt, wt, bb, xd, g, scr_eng):
                xp = psx.tile([128, D], F32, tag="xmm")
                nc.tensor.matmul(xp[:], xt[0][:, g * 128:(g + 1) * 128], wt[0][:],
                                 start=True, stop=False)
                nc.tensor.matmul(xp[:], xt[1][:, g * 128:(g + 1) * 128], wt[1][:],
                                 start=False, stop=False)
                nc.tensor.matmul(xp[:], ones1[0:1, :], bb[:], start=False, stop=True)
                xs = sbin.tile([128, D], BF16, tag=f"xsb{g}")
                nc.vector.tensor_copy(xs[:], xp[:])
                scr_eng.dma_start(xd[g * 128:(g + 1) * 128, :], xs[:])

            # merged gather per (side, graph): xfg [128=(n,ss), 256=(hq,pp,k)]
            xfg1, xfg2 = [None] * GSH, [None] * GSH
            x1t, x2t = [None] * NSTACK, [None] * NSTACK

            def gather_graph(g, eng1, eng2):
                for (xd, store, nm, eng) in ((x1_dram, xfg1, "x1", eng1),
                                             (x2_dram, xfg2, "x2", eng2)):
                    xfg = sbt.tile([128, 2 * 128], BF16, tag=f"{nm}fg{g}")
                    srcv = xd[:].rearrange("(g2 hq pp n) (ss k) -> g2 n ss hq pp k",
                                           g2=GSH, hq=2, pp=SPP, ss=H)
                    eng.dma_start(xfg[:].rearrange("p (hq ppk) -> p hq ppk", hq=2),
                                  srcv[g])
                    store[g] = xfg

            def trans_stack(s):
                g, hq = s // 2, s % 2
                # x1t: quad-broadcast transpose [(pp,k), 4*i]; x2t: [(pp,k), j]
                tp1 = pstb.tile([128, 128], BF16, tag="trb")
                nc.tensor.transpose(tp1[:], xfg1[g][:, hq * 128:(hq + 1) * 128],
                                    ident_bf[:])
                xtb = sbtr.tile([128, 4 * 128], BF16, tag="x1tq")
                nc.vector.tensor_copy(xtb[:].rearrange("p (i q) -> p i q", q=4),
                                      tp1[:].unsqueeze(2).broadcast_to([128, 128, 4]))
                x1t[s] = xtb
                tp2 = pstb.tile([128, 128], BF16, tag="trb")
                nc.tensor.transpose(tp2[:], xfg2[g][:, hq * 128:(hq + 1) * 128],
                                    ident_bf[:])
                x2tb = sbtr.tile([128, 128], BF16, tag="x2tp")
                nc.vector.tensor_copy(x2tb[:], tp2[:])
                x2t[s] = x2tb

            # ---- g0 chain: weights -> xmm -> scratch -> gather -> transposes ----
            trans_block(blk_w1_0, w1t, 0, use_act=True)
            trans_block(blk_w1_1, w1t, 1, use_act=True)
            trans_block(blk_a_0, at, 0)
            xmm(at, w1t, b1_bf, x1_dram, 0, nc.gpsimd)
            trans_block(blk_w2_0, w2t, 0, use_act=True)
            trans_block(blk_w2_1, w2t, 1, use_act=True)
            trans_block(blk_b_0, bt, 0)
            xmm(bt, w2t, b2_bf, x2_dram, 0, nc.sync)
            gather_graph(0, nc.gpsimd, nc.sync)
            # qdiag [128, SPP]: qdiag[(pp,k), pp'] = q[k] * (pp == pp')
            for pp in range(SPP):
                nc.sync.dma_start(qdiag[pp * NK:(pp + 1) * NK, pp:pp + 1], q_bf[:])
            trans_stack(0)

            # ------------------------- stack compute -------------------------
            probs_at = sm.tile([128, NPAIR], BF16, tag="pta")
            probs_bt = sm.tile([128, NPAIR], BF16, tag="ptb")

            def soft_half(s, ex, half, pt, g, hq):
                """softmax normalize ex[:, half*NJ:...], transpose, project, out."""
                sme = sm.tile([SPP, 1], F32, tag=f"sm{half}")
                nc.vector.tensor_reduce(sme[:], ex[:, half * NJ:(half + 1) * NJ],
                                        axis=mybir.AxisListType.X,
                                        op=mybir.AluOpType.add)
                rcp = sm.tile([SPP, 1], F32, tag=f"rc{half}")
                nc.vector.reciprocal(rcp[:], sme[:])
                pr = sm.tile([SPP, NJ], F32, tag=f"pr{half}")
                nc.vector.tensor_scalar_mul(pr[:], ex[:, half * NJ:(half + 1) * NJ],
                                            rcp[:])
                pp_ps = pst.tile([128, SPP], F32, tag="tr")
                nc.tensor.transpose(pp_ps[:], pr[:], ident[0:SPP, 0:SPP])
                nc.vector.tensor_copy(pt[:, s * SPP:(s + 1) * SPP], pp_ps[:])
                xfg = (xfg1 if half == 0 else xfg2)[g]
                pj = psproj.tile([128, SPP], F32, tag="proj")
                nc.tensor.matmul(pj[:], xfg[:, hq * 128:(hq + 1) * 128],
                                 pt[:, s * SPP:(s + 1) * SPP],
                                 start=True, stop=True)
                pjs = sm.tile([NK, SPP], F32, tag=f"projs{s}_{half}")
                for pp in range(SPP):
                    nc.vector.tensor_copy(pjs[:, pp:pp + 1],
                                          pj[pp * NK:(pp + 1) * NK, pp:pp + 1])
                dst = out_d[g:g + 1, half * D + hq * SPP * HD:
                            half * D + (hq + 1) * SPP * HD]
                dst = dst.rearrange("o (pp k) -> o k pp", pp=SPP)
                nc.sync.dma_start(dst, pjs[:])

            def epilogue(s, a2b_ps, b2a_ps, lgt, split=False):
                """a2b reduce + softmax + both projections (b2a logits already
                reduced per-chunk into lgt[:, 0:NJ]). split=True exps the two
                halves separately so the a2b side finishes without waiting for
                the fold-gated b2a side (shrinks the final drain)."""
                g, hq = s // 2, s % 2
                nc.vector.tensor_reduce(
                    lgt[:, NJ:2 * NJ], a2b_ps[:].rearrange("p (io j) -> p j io", io=4),
                    axis=mybir.AxisListType.X, op=mybir.AluOpType.add)
                ex = sm.tile([SPP, 2 * NJ], F32, tag="ex")
                if split:
                    nc.scalar.activation(ex[:, NJ:2 * NJ], lgt[:, NJ:2 * NJ],
                                         mybir.ActivationFunctionType.Exp,
                                         scale=1.0 / NJ)
                    soft_half(s, ex, 1, probs_bt, g, hq)
                    nc.scalar.activation(ex[:, 0:NJ], lgt[:, 0:NJ],
                                         mybir.ActivationFunctionType.Exp,
                                         scale=1.0 / NJ)
                    soft_half(s, ex, 0, probs_at, g, hq)
                else:
                    nc.scalar.activation(ex[:], lgt[:],
                                         mybir.ActivationFunctionType.Exp,
                                         scale=1.0 / NJ)
                    soft_half(s, ex, 1, probs_bt, g, hq)
                    soft_half(s, ex, 0, probs_at, g, hq)

            pending = [None]

            def stack(s):
                g, hq = s // 2, s % 2
                last = s == NSTACK - 1
                p4 = big.tile([128, FF], BF16, tag="p4")
                t4 = big.tile([128, FF], BF16, tag="t4")
                a2b_ps = psacc.tile([SPP, 4 * NJ], F32, tag="acc")
                b2a_ps = None
                lgt = sm.tile([SPP, 2 * NJ], F32, tag="lgt")
                io = 0
                nmm = FF // (4 * NJ)    # 32 a2b matmuls per stack
                for ci, w in enumerate(WIDTHS[s]):
                    # DVE: p4 chunk = x1 (x) x2 (broadcast quad layout)
                    in0 = x1t[s][:, io * 4:(io + w) * 4].rearrange("p (i q) -> p i q", q=4)\
                        .unsqueeze(2).broadcast_to([128, w, NJ // 4, 4])
                    in1 = x2t[s][:].rearrange("p (j2 q) -> p j2 q", q=4)\
                        .unsqueeze(1).broadcast_to([128, w, NJ // 4, 4])
                    nc.vector.tensor_tensor(
                        p4[:, io * NJ:(io + w) * NJ].rearrange("p (i j2 q) -> p i j2 q",
                                                               q=4, j2=NJ // 4),
                        in0, in1, op=mybir.AluOpType.mult)
                    # ACT: tanh chunk
                    nc.scalar.activation(t4[:, io * NJ:(io + w) * NJ],
                                         p4[:, io * NJ:(io + w) * NJ],
                                         mybir.ActivationFunctionType.Tanh)
                    # PE: a2b accumulation (i-sum via PSUM) over 512-col groups
                    for u in range(w * NJ // (4 * NJ)):
                        ch = io // 4 + u
                        nc.tensor.matmul(a2b_ps[:], qdiag[:],
                                         t4[:, ch * 4 * NJ:(ch + 1) * 4 * NJ],
                                         start=(ch == 0), stop=(ch == nmm - 1))
                    # fold chunk over j by contiguous halves: 128 -> 64 -> 32 -> 16
                    # (last stack's trailing chunks fold on DVE: Pool backlog
                    #  otherwise gates the final b2a -> softmax -> out chain)
                    feng = nc.vector if (last and ci >= 4) else nc.gpsimd
                    jf1 = jfp.tile([128, 64 * (NJ // 2)], BF16, tag="jf1")
                    jf2 = jf2p.tile([128, 64 * (NJ // 4)], BF16, tag="jf2")
                    jf3 = jf3p.tile([128, 64 * (NJ // 8)], BF16, tag="jf3")
                    vt = t4[:, io * NJ:(io + w) * NJ].rearrange("p (i j) -> p i j", j=NJ)
                    v0 = jf1[:, :w * (NJ // 2)].rearrange("p (i j) -> p i j", j=NJ // 2)
                    feng.tensor_tensor(v0, vt[:, :, 0:NJ // 2], vt[:, :, NJ // 2:NJ],
                                       op=mybir.AluOpType.add)
                    v1 = jf2[:, :w * (NJ // 4)].rearrange("p (i j) -> p i j", j=NJ // 4)
                    nc.vector.tensor_tensor(v1, v0[:, :, 0:NJ // 4], v0[:, :, NJ // 4:NJ // 2],
                                            op=mybir.AluOpType.add)
                    v2 = jf3[:, :w * (NJ // 8)].rearrange("p (i j) -> p i j", j=NJ // 8)
                    nc.vector.tensor_tensor(v2, v1[:, :, 0:NJ // 8], v1[:, :, NJ // 8:NJ // 4],
                                            op=mybir.AluOpType.add)
                    # PE: b2a accumulation for this chunk (j-sum via PSUM)
                    if b2a_ps is None:
                        b2a_ps = psacc.tile([SPP, 4 * NJ], F32, tag="acc")
                    for jq in range(NJ // 8 // 4):
                        nc.tensor.matmul(
                            b2a_ps[:, io * 4:(io + w) * 4].rearrange("p (i j) -> p i j", j=4),
                            qdiag[:], v2[:, :, 4 * jq:4 * (jq + 1)],
                            start=(jq == 0), stop=(jq == NJ // 8 // 4 - 1))
                    # DVE: per-chunk partial b2a logit reduce -> lgt[:, io:io+w]
                    nc.vector.tensor_reduce(
                        lgt[:, io:io + w],
                        b2a_ps[:, io * 4:(io + w) * 4].rearrange("p (i j) -> p i j", j=4),
                        axis=mybir.AxisListType.X, op=mybir.AluOpType.add)

                    if ci == 0:
                        # deferred work rides inside this stack's tanh window
                        if pending[0] is not None:
                            pending[0]()
                            pending[0] = None
                        if s == 0:
                            trans_stack(1)
                    if ci == 2:
                        if s == 0:
                            blk_b_1 = load_block(b_d, 1, "B", nc.gpsimd)
                            blk_a_1 = load_block(a_d, 1, "A", nc.gpsimd)
                            trans_block(blk_b_1, bt, 1)
                            xmm(bt, w2t, b2_bf, x2_dram, 1, nc.gpsimd)
                            trans_block(blk_a_1, at, 1)
                            xmm(at, w1t, b1_bf, x1_dram, 1, nc.gpsimd)
                            gather_graph(1, nc.sync, nc.sync)
                        if s == 1:
                            trans_stack(2)
                            trans_stack(3)
                    io += w

                if last:
                    epilogue(s, a2b_ps, b2a_ps, lgt, split=True)
                else:
                    pending[0] = lambda: epilogue(s, a2b_ps, b2a_ps, lgt)

            for s in range(NSTACK):
                stack(s)

        if reps == 1:
            body()
        else:
            with tc.For_i(0, reps):
                body()

    if not nc.is_finalized():
        nc.finalize()
    return nc


def shard_inputs(inputs):
    """Full inputs -> list of 8 per-core input maps."""
    A = np.asarray(inputs["A"], np.float32)
    B = np.asarray(inputs["B"], np.float32)
    maps = []
    for c in range(NCORES):
        maps.append({
            "A": np.ascontiguousarray(A[c * GSH * NA:(c + 1) * GSH * NA]),
            "B": np.ascontiguousarray(B[c * GSH * NB:(c + 1) * GSH * NB]),
            "W1": np.asarray(inputs["W1"], np.float32),
            "W2": np.asarray(inputs["W2"], np.float32),
            "bias1": np.asarray(inputs["bias1"], np.float32).reshape(1, D),
            "bias2": np.asarray(inputs["bias2"], np.float32).reshape(1, D),
            "q": np.asarray(inputs["q"], np.float32).reshape(1, NK),
        })
    return maps


_NC_CACHE = {}


def kernel(**inputs) -> np.ndarray:
    """Full (unsharded) inputs -> full [G, 2D] output, running on 8 cores."""
    from concourse.bass_utils import run_bass_kernel_spmd

    if "nc" not in _NC_CACHE:
        _NC_CACHE["nc"] = build_kernel()
    nc = _NC_CACHE["nc"]
    in_maps = shard_inputs(inputs)
    res = run_bass_kernel_spmd(nc, in_maps, core_ids=list(range(NCORES)))
    out = np.concatenate([res.results[c]["out"] for c in range(NCORES)], axis=0)
    return out.astype(np.float32)


def _ref_core(m):
    x1 = m["A"] @ m["W1"].T + m["bias1"][0]
    x2 = m["B"] @ m["W2"].T + m["bias2"][0]
    x1 = x1.reshape(GSH, H, NA, HD)
    x2 = x2.reshape(GSH, H, NB, HD)
    att = np.einsum("ghijk,k->ghij",
                    np.tanh(x1[:, :, :, None, :] * x2[:, :, None, :, :]), m["q"][0])

    def smax(v, ax):
        v = v - v.max(axis=ax, keepdims=True)
        e = np.exp(v)
        return e / e.sum(axis=ax, keepdims=True)

    b2a = smax(att.mean(axis=3), 2)
    a2b = smax(att.mean(axis=2), 2)
    A_p = np.einsum("ghik,ghi->ghk", x1, b2a).reshape(GSH, D)
    B_p = np.einsum("ghjk,ghj->ghk", x2, a2b).reshape(GSH, D)
    return np.concatenate([A_p, B_p], axis=1)


if __name__ == "__main__":
    from concourse.bass_interp import CoreSim

    reps = int(sys.argv[1]) if len(sys.argv) > 1 else 1
    trace = len(sys.argv) > 2 and sys.argv[2] == "trace"
    rng = np.random.default_rng(0)
    scale = 1.0 / np.sqrt(D)
    full = {
        "A": rng.standard_normal((G * NA, D)).astype(np.float32),
        "B": rng.standard_normal((G * NB, D)).astype(np.float32),
        "W1": (rng.standard_normal((D, D)) * scale).astype(np.float32),
        "bias1": (rng.standard_normal(D) * scale).astype(np.float32),
        "W2": (rng.standard_normal((D, D)) * scale).astype(np.float32),
        "bias2": (rng.standard_normal(D) * scale).astype(np.float32),
        "q": (rng.standard_normal(HD) * scale).astype(np.float32),
    }

    nc = build_kernel(reps=reps)
    m0 = shard_inputs(full)[0]
    sim = CoreSim(nc, trace=trace)
    for k, v in m0.items():
        sim.tensor(k)[:] = v
    sim.simulate()
    got = sim.tensor("out").copy()
    want = _ref_core(m0)
    err = np.abs(got - want).max() / np.abs(want).max()
    print("sim time:", sim.time, "ns", f"({reps} reps)")
    print("rel err:", err)


# revision 7
# speedup vs baseline: 1567.3721x; 1.5494x over previous
"""Trainium2 Bass kernel for nn_BilinearAttention2 (gnn_message_passing).

Math (per graph g, head h; head h = raw C-order view of x[g] [128,256] as
[8,128,32], i.e. 16 consecutive rows -> 128 pseudo-nodes x 32 dims):
  x1 = A @ W1.T + b1 ; x2 = B @ W2.T + b2
  att[i,j]  = sum_k tanh(X1[i,k] * X2[j,k]) * q[k]
  b2a = softmax_i(mean_j att); a2b = softmax_j(mean_i att)
  A_p[g,h] = X1.T @ b2a ; B_p[g,h] = X2.T @ a2b ; out[g] = [A_p | B_p]

Sharding: data-parallel over graphs: 8 cores x 2 graphs. Per core 16 (g,h)
pairs processed as 4 stacks of 4: partition = (pair, k) = (4,32)=128,
free = (i,j) = 16384 per stack.

Engine plan (HW-measured; real GPSIMD is ~3-5x slower than the CoreSim
model, so Pool does no bulk work):
  DVE : broadcast mult p4 = x1 (x) x2 (bf16, 2x mode)  ~8.8us/stack
  ACT : t4 = tanh(p4) -- the hard bottleneck (1 elem/cycle/partition)
  PE  : BOTH reductions as accumulating qdiag matmuls over t4:
        a2b (i-sum) via contiguous 512-col chunks; b2a (j-sum) via strided
        j-quad rhs per half-stack. PSUM accumulation does the sums free.
  DVE : [4,512] strided PSUM reduces in each stack's own slack, batched-Exp
        softmax, projections. Last stack uses a small all-DVE fold chain and
        drains the a2b side early to shorten the tail.
Program order software-pipelines each stack's epilogue and the graph-1
prologue into the next stack's tanh window; chunk widths taper at both ends.

build_kernel(reps=N) wraps the whole per-call body in a hardware loop:
e2e(reps hi) - e2e(reps lo) isolates true per-iteration HW time from the
~68-100ms axon dispatch floor.
"""
import sys

sys.path.insert(0, "/opt/trn_rl_repo")

from contextlib import ExitStack

import numpy as np

import concourse.bass as bass
import concourse.bacc as bacc
import concourse.mybir as mybir
import concourse.tile as tile
from concourse.masks import make_identity

F32 = mybir.dt.float32
BF16 = mybir.dt.bfloat16

D = 256
H = 8
HD = 32
G = 16
NA = 128
NB = 128
NCORES = 8
GSH = G // NCORES          # graphs per core = 2
NPAIR = GSH * H            # 16 (g,h) pairs per core
SPP = 4                    # pairs per stack
NSTACK = NPAIR // SPP      # 4
NK = HD                    # 32
NJ = 128                   # nodes per head-view
FF = NJ * NJ               # 16384 free elems per stack

WIDTHS = [
    (16, 16, 32, 32, 32),   # stack 0: small first chunks -> ACT starts early
    (32, 32, 32, 32),
    (32, 32, 32, 32),
    (32, 32, 32, 24, 8),    # stack 3: small last chunk -> short drain tail
]


def build_kernel(reps: int = 1):
    nc = bacc.Bacc()
    a_d = nc.dram_tensor("A", [GSH * NA, D], F32, kind="ExternalInput")
    b_d = nc.dram_tensor("B", [GSH * NB, D], F32, kind="ExternalInput")
    w1_d = nc.dram_tensor("W1", [D, D], F32, kind="ExternalInput")
    w2_d = nc.dram_tensor("W2", [D, D], F32, kind="ExternalInput")
    b1_d = nc.dram_tensor("bias1", [1, D], F32, kind="ExternalInput")
    b2_d = nc.dram_tensor("bias2", [1, D], F32, kind="ExternalInput")
    q_d = nc.dram_tensor("q", [1, NK], F32, kind="ExternalInput")
    out_d = nc.dram_tensor("out", [GSH, 2 * D], F32, kind="ExternalOutput")

    # DRAM scratch for the head-view gather (bf16: half the traffic)
    x1_dram = nc.dram_tensor("x1_scratch", [GSH * NA, D], BF16)
    x2_dram = nc.dram_tensor("x2_scratch", [GSH * NB, D], BF16)

    with tile.TileContext(nc) as tc, ExitStack() as ctx:
        cst = ctx.enter_context(tc.tile_pool(name="cst", bufs=1))
        sbin = ctx.enter_context(tc.tile_pool(name="sbin", bufs=1))
        sbt = ctx.enter_context(tc.tile_pool(name="sbt", bufs=1))
        sbtr = ctx.enter_context(tc.tile_pool(name="sbtr", bufs=2))
        big = ctx.enter_context(tc.tile_pool(name="big", bufs=2))
        jfp = ctx.enter_context(tc.tile_pool(name="jfp", bufs=2))
        jf2p = ctx.enter_context(tc.tile_pool(name="jf2p", bufs=2))
        jf3p = ctx.enter_context(tc.tile_pool(name="jf3p", bufs=2))
        sm = ctx.enter_context(tc.tile_pool(name="sm", bufs=2))
        pst = ctx.enter_context(tc.tile_pool(name="pst", bufs=2, space="PSUM"))
        pstb = ctx.enter_context(tc.tile_pool(name="pstb", bufs=1, space="PSUM"))
        psx = ctx.enter_context(tc.tile_pool(name="psx", bufs=1, space="PSUM"))
        psacc = ctx.enter_context(tc.tile_pool(name="psacc", bufs=3, space="PSUM"))
        psproj = ctx.enter_context(tc.tile_pool(name="psproj", bufs=1, space="PSUM"))

        ident = cst.tile([128, 128], F32)
        make_identity(nc, ident[:])
        ident_bf = cst.tile([128, 128], BF16)
        nc.vector.tensor_copy(ident_bf[:], ident[:])
        ones1 = cst.tile([1, 128], BF16)
        nc.vector.memset(ones1[:], 1.0)

        def body():
            # ---------------- prologue: loads + weight transposes ----------------
            def load_block(src_d, r, name, eng):
                blk = sbin.tile([128, D], F32, tag=f"{name}ld")
                eng.dma_start(blk[:], src_d[r * 128:(r + 1) * 128, :])
                return blk

            def trans_block(blk, t_sb, r, use_act=False):
                # NB: Pool/GPSIMD cannot touch PSUM on real HW (BIR verifier),
                # so PSUM->SBUF copies go to DVE or, in the idle prologue, ACT.
                for c in range(2):
                    tp = pst.tile([128, 128], F32, tag="tr")
                    nc.tensor.transpose(tp[:], blk[:, c * 128:(c + 1) * 128], ident[:])
                    if use_act:
                        nc.scalar.copy(t_sb[c][:, r * 128:(r + 1) * 128], tp[:])
                    else:
                        nc.vector.tensor_copy(t_sb[c][:, r * 128:(r + 1) * 128], tp[:])

            def alloc_t(name):
                return [sbin.tile([128, D], BF16, tag=f"{name}T{c}", name=f"{name}T{c}")
                        for c in range(2)]

            at, bt, w1t, w2t = alloc_t("A"), alloc_t("B"), alloc_t("W1"), alloc_t("W2")
            b1_sb = sbin.tile([1, D], F32)
            b2_sb = sbin.tile([1, D], F32)
            b1_bf = sbin.tile([1, D], BF16, tag="b1bf")
            b2_bf = sbin.tile([1, D], BF16, tag="b2bf")
            q_sb = sbin.tile([1, NK], F32, tag="q")
            q_bf = sbin.tile([1, NK], BF16, tag="qbf")
            qdiag = sbin.tile([128, SPP], BF16, tag="qdiag")

            # parallel dispatch of g0-critical loads across non-ACT queues
            blk_w1_0 = load_block(w1_d, 0, "W1", nc.sync)
            blk_w1_1 = load_block(w1_d, 1, "W1b", nc.sync)
            blk_a_0 = load_block(a_d, 0, "A", nc.gpsimd)
            nc.gpsimd.dma_start(b1_sb[:], b1_d[:])
            nc.sync.dma_start(b2_sb[:], b2_d[:])
            nc.gpsimd.dma_start(q_sb[:], q_d[:])
            blk_w2_0 = load_block(w2_d, 0, "W2", nc.gpsimd)
            blk_w2_1 = load_block(w2_d, 1, "W2b", nc.sync)
            blk_b_0 = load_block(b_d, 0, "B", nc.gpsimd)
            nc.gpsimd.tensor_copy(b2_bf[:], b2_sb[:])
            nc.gpsimd.tensor_copy(b1_bf[:], b1_sb[:])
            nc.gpsimd.tensor_copy(q_bf[:], q_sb[:])
            nc.gpsimd.memset(qdiag[:], 0.0)

            def xmm(xt, wt, bb, xd, g, scr_eng):
                xp = psx.tile([128, D], F32, tag="xmm")
                nc.tensor.matmul(xp[:], xt[0][:, g * 128:(g + 1) * 128], wt[0][:],
                                 start=True, stop=False)
                nc.tensor.matmul(xp[:], xt[1][:, g * 128:(g + 1) * 128], wt[1][:],
                                 start=False, stop=False)
                nc.tensor.matmul(xp[:], ones1[0:1, :], bb[:], start=False, stop=True)
                xs = sbin.tile([128, D], BF16, tag=f"xsb{g}")
                nc.vector.tensor_copy(xs[:], xp[:])
                scr_eng.dma_start(xd[g * 128:(g + 1) * 128, :], xs[:])

            # merged gather per (side, graph): xfg [128=(n,ss), 256=(hq,pp,k)]
            xfg1, xfg2 = [None] * GSH, [None] * GSH
            x1t, x2t = [None] * NSTACK, [None] * NSTACK

            def gather_graph(g, eng1, eng2):
                for (xd, store, nm, eng) in ((x1_dram, xfg1, "x1", eng1),
                                             (x2_dram, xfg2, "x2", eng2)):
                    xfg = sbt.tile([128, 2 * 128], BF16, tag=f"{nm}fg{g}")
                    srcv = xd[:].rearrange("(g2 hq pp n) (ss k) -> g2 n ss hq pp k",
                                           g2=GSH, hq=2, pp=SPP, ss=H)
                    eng.dma_start(xfg[:].rearrange("p (hq ppk) -> p hq ppk", hq=2),
                                  srcv[g])
                    store[g] = xfg

            def trans_stack(s):
                g, hq = s // 2, s % 2
                # x1t: quad-broadcast transpose [(pp,k), 4*i]; x2t: [(pp,k), j]
                tp1 = pstb.tile([128, 128], BF16, tag="trb")
                nc.tensor.transpose(tp1[:], xfg1[g][:, hq * 128:(hq + 1) * 128],
                                    ident_bf[:])
                xtb = sbtr.tile([128, 4 * 128], BF16, tag="x1tq")
                nc.vector.tensor_copy(xtb[:].rearrange("p (i q) -> p i q", q=4),
                                      tp1[:].unsqueeze(2).broadcast_to([128, 128, 4]))
                x1t[s] = xtb
                tp2 = pstb.tile([128, 128], BF16, tag="trb")
                nc.tensor.transpose(tp2[:], xfg2[g][:, hq * 128:(hq + 1) * 128],
                                    ident_bf[:])
                x2tb = sbtr.tile([128, 128], BF16, tag="x2tp")
                nc.vector.tensor_copy(x2tb[:], tp2[:])
                x2t[s] = x2tb

            # ---- g0 chain: weights -> xmm -> scratch -> gather -> transposes ----
            trans_block(blk_w1_0, w1t, 0, use_act=True)
            trans_block(blk_w1_1, w1t, 1, use_act=True)
            trans_block(blk_a_0, at, 0)
            xmm(at, w1t, b1_bf, x1_dram, 0, nc.gpsimd)
            trans_block(blk_w2_0, w2t, 0, use_act=True)
            trans_block(blk_w2_1, w2t, 1, use_act=True)
            trans_block(blk_b_0, bt, 0)
            xmm(bt, w2t, b2_bf, x2_dram, 0, nc.sync)
            gather_graph(0, nc.gpsimd, nc.sync)
            # qdiag [128, SPP]: qdiag[(pp,k), pp'] = q[k] * (pp == pp')
            for pp in range(SPP):
                nc.sync.dma_start(qdiag[pp * NK:(pp + 1) * NK, pp:pp + 1], q_bf[:])
            trans_stack(0)

            # ------------------------- stack compute -------------------------
            probs_at = sm.tile([128, NPAIR], BF16, tag="pta")
            probs_bt = sm.tile([128, NPAIR], BF16, tag="ptb")

            def soft_half(s, ex, half, pt, g, hq):
                """softmax normalize ex[:, half*NJ:...], transpose, project, out."""
                sme = sm.tile([SPP, 1], F32, tag=f"sm{half}")
                nc.vector.tensor_reduce(sme[:], ex[:, half * NJ:(half + 1) * NJ],
                                        axis=mybir.AxisListType.X,
                                        op=mybir.AluOpType.add)
                rcp = sm.tile([SPP, 1], F32, tag=f"rc{half}")
                nc.vector.reciprocal(rcp[:], sme[:])
                pr = sm.tile([SPP, NJ], F32, tag=f"pr{half}")
                nc.vector.tensor_scalar_mul(pr[:], ex[:, half * NJ:(half + 1) * NJ],
                                            rcp[:])
                pp_ps = pst.tile([128, SPP], F32, tag="tr")
                nc.tensor.transpose(pp_ps[:], pr[:], ident[0:SPP, 0:SPP])
                nc.vector.tensor_copy(pt[:, s * SPP:(s + 1) * SPP], pp_ps[:])
                xfg = (xfg1 if half == 0 else xfg2)[g]
                pj = psproj.tile([128, SPP], F32, tag="proj")
                nc.tensor.matmul(pj[:], xfg[:, hq * 128:(hq + 1) * 128],
                                 pt[:, s * SPP:(s + 1) * SPP],
                                 start=True, stop=True)
                pjs = sm.tile([NK, SPP], F32, tag=f"projs{s}_{half}")
                for pp in range(SPP):
                    nc.vector.tensor_copy(pjs[:, pp:pp + 1],
                                          pj[pp * NK:(pp + 1) * NK, pp:pp + 1])
                dst = out_d[g:g + 1, half * D + hq * SPP * HD:
                            half * D + (hq + 1) * SPP * HD]
                dst = dst.rearrange("o (pp k) -> o k pp", pp=SPP)
                nc.sync.dma_start(dst, pjs[:])

            def epi_b_side(s, a2b_ps, lgt, ex):
                g, hq = s // 2, s % 2
                nc.vector.tensor_reduce(
                    lgt[:, NJ:2 * NJ], a2b_ps[:].rearrange("p (io j) -> p j io", io=4),
                    axis=mybir.AxisListType.X, op=mybir.AluOpType.add)
                nc.scalar.activation(ex[:, NJ:2 * NJ], lgt[:, NJ:2 * NJ],
                                     mybir.ActivationFunctionType.Exp, scale=1.0 / NJ)
                soft_half(s, ex, 1, probs_bt, g, hq)

            def epilogue(s, a2b_ps, b2a_ps, lgt, split=False):
                """a2b reduce + softmax + both projections (b2a logits already
                reduced per-chunk into lgt[:, 0:NJ]). split=True exps the two
                halves separately so the a2b side finishes without waiting for
                the fold-gated b2a side (shrinks the final drain)."""
                g, hq = s // 2, s % 2
                if not split:
                    nc.vector.tensor_reduce(
                        lgt[:, NJ:2 * NJ],
                        a2b_ps[:].rearrange("p (io j) -> p j io", io=4),
                        axis=mybir.AxisListType.X, op=mybir.AluOpType.add)
                ex = ex_last[0] if split else sm.tile([SPP, 2 * NJ], F32, tag="ex")
                if split:
                    # B side already emitted early via epi_b_side
                    nc.scalar.activation(ex[:, 0:NJ], lgt[:, 0:NJ],
                                         mybir.ActivationFunctionType.Exp,
                                         scale=1.0 / NJ)
                    soft_half(s, ex, 0, probs_at, g, hq)
                else:
                    nc.scalar.activation(ex[:], lgt[:],
                                         mybir.ActivationFunctionType.Exp,
                                         scale=1.0 / NJ)
                    soft_half(s, ex, 1, probs_bt, g, hq)
                    soft_half(s, ex, 0, probs_at, g, hq)

            pending = [None]
            ex_last = [None]

            def stack(s):
                g, hq = s // 2, s % 2
                last = s == NSTACK - 1
                p4 = big.tile([128, FF], BF16, tag="p4")
                t4 = big.tile([128, FF], BF16, tag="t4")
                a2b_ps = psacc.tile([SPP, 4 * NJ], F32, tag="acc")
                b2a_ps = None
                lgt = sm.tile([SPP, 2 * NJ], F32, tag="lgt")
                io = 0
                nmm = FF // (4 * NJ)    # 32 a2b matmuls per stack
                for ci, w in enumerate(WIDTHS[s]):
                    # DVE: p4 chunk = x1 (x) x2 (broadcast quad layout)
                    in0 = x1t[s][:, io * 4:(io + w) * 4].rearrange("p (i q) -> p i q", q=4)\
                        .unsqueeze(2).broadcast_to([128, w, NJ // 4, 4])
                    in1 = x2t[s][:].rearrange("p (j2 q) -> p j2 q", q=4)\
                        .unsqueeze(1).broadcast_to([128, w, NJ // 4, 4])
                    nc.vector.tensor_tensor(
                        p4[:, io * NJ:(io + w) * NJ].rearrange("p (i j2 q) -> p i j2 q",
                                                               q=4, j2=NJ // 4),
                        in0, in1, op=mybir.AluOpType.mult)
                    # ACT: tanh chunk
                    nc.scalar.activation(t4[:, io * NJ:(io + w) * NJ],
                                         p4[:, io * NJ:(io + w) * NJ],
                                         mybir.ActivationFunctionType.Tanh)
                    # PE: a2b accumulation (i-sum via PSUM) over 512-col groups
                    for u in range(w * NJ // (4 * NJ)):
                        ch = io // 4 + u
                        nc.tensor.matmul(a2b_ps[:], qdiag[:],
                                         t4[:, ch * 4 * NJ:(ch + 1) * 4 * NJ],
                                         start=(ch == 0), stop=(ch == nmm - 1))
                    if b2a_ps is None:
                        b2a_ps = psacc.tile([SPP, 4 * NJ], F32, tag="acc")
                    if last and ci == len(WIDTHS[s]) - 1:
                        # drain the a2b (B) side now: its PE accumulation just
                        # finished; runs in parallel with the fold-gated A side
                        ex_last[0] = sm.tile([SPP, 2 * NJ], F32, tag="ex", name="ex_s3")
                        epi_b_side(s, a2b_ps, lgt, ex_last[0])
                    if not last:
                        # b2a j-sum DIRECTLY on PE: when a half-stack of tanh is
                        # done, stream it through qdiag with strided j-quad rhs,
                        # accumulating the j-sum in PSUM (no folds: Pool runs
                        # ~3x slower on real HW than the cost model claims)
                        if io + w in (NJ // 2, NJ):
                            hb = 0 if io + w == NJ // 2 else NJ // 2
                            vh = t4[:, hb * NJ:(hb + NJ // 2) * NJ].rearrange(
                                "p (i j) -> p i j", j=NJ)
                            for jq in range(NJ // 4):
                                nc.tensor.matmul(
                                    b2a_ps[:, hb * 4:(hb + NJ // 2) * 4].rearrange(
                                        "p (i j) -> p i j", j=4),
                                    qdiag[:], vh[:, :, 4 * jq:4 * (jq + 1)],
                                    start=(jq == 0), stop=(jq == NJ // 4 - 1))
                            # partial b2a logit reduce inside this stack's own
                            # window (DVE slack here; keeps the deferred
                            # epilogue off the next stack's critical DVE path)
                            nc.vector.tensor_reduce(
                                lgt[:, hb:hb + NJ // 2],
                                b2a_ps[:, hb * 4:(hb + NJ // 2) * 4].rearrange(
                                    "p (i j) -> p i j", j=4),
                                axis=mybir.AxisListType.X, op=mybir.AluOpType.add)
                    else:
                        # last stack: all-DVE fold chain keeps the drain short
                        jf1 = jfp.tile([128, 64 * (NJ // 2)], BF16, tag="jf1")
                        jf2 = jf2p.tile([128, 64 * (NJ // 4)], BF16, tag="jf2")
                        jf3 = jf3p.tile([128, 64 * (NJ // 8)], BF16, tag="jf3")
                        vt = t4[:, io * NJ:(io + w) * NJ].rearrange("p (i j) -> p i j", j=NJ)
                        v0 = jf1[:, :w * (NJ // 2)].rearrange("p (i j) -> p i j", j=NJ // 2)
                        nc.vector.tensor_tensor(v0, vt[:, :, 0:NJ // 2], vt[:, :, NJ // 2:NJ],
                                                op=mybir.AluOpType.add)
                        v1 = jf2[:, :w * (NJ // 4)].rearrange("p (i j) -> p i j", j=NJ // 4)
                        nc.vector.tensor_tensor(v1, v0[:, :, 0:NJ // 4],
                                                v0[:, :, NJ // 4:NJ // 2],
                                                op=mybir.AluOpType.add)
                        v2 = jf3[:, :w * (NJ // 8)].rearrange("p (i j) -> p i j", j=NJ // 8)
                        nc.vector.tensor_tensor(v2, v1[:, :, 0:NJ // 8],
                                                v1[:, :, NJ // 8:NJ // 4],
                                                op=mybir.AluOpType.add)
                        for jq in range(NJ // 8 // 4):
                            nc.tensor.matmul(
                                b2a_ps[:, io * 4:(io + w) * 4].rearrange("p (i j) -> p i j", j=4),
                                qdiag[:], v2[:, :, 4 * jq:4 * (jq + 1)],
                                start=(jq == 0), stop=(jq == NJ // 8 // 4 - 1))
                        nc.vector.tensor_reduce(
                            lgt[:, io:io + w],
                            b2a_ps[:, io * 4:(io + w) * 4].rearrange("p (i j) -> p i j", j=4),
                            axis=mybir.AxisListType.X, op=mybir.AluOpType.add)

                    if ci == 0:
                        # deferred work rides inside this stack's tanh window
                        if pending[0] is not None:
                            pending[0]()
                            pending[0] = None
                        if s == 0:
                            trans_stack(1)
                    if ci == 2:
                        if s == 0:
                            blk_b_1 = load_block(b_d, 1, "B", nc.gpsimd)
                            blk_a_1 = load_block(a_d, 1, "A", nc.gpsimd)
                            trans_block(blk_b_1, bt, 1)
                            xmm(bt, w2t, b2_bf, x2_dram, 1, nc.gpsimd)
                            trans_block(blk_a_1, at, 1)
                            xmm(at, w1t, b1_bf, x1_dram, 1, nc.gpsimd)
                            gather_graph(1, nc.sync, nc.sync)
                        if s == 1:
                            trans_stack(2)
                            trans_stack(3)
                    io += w

                if last:
                    epilogue(s, a2b_ps, b2a_ps, lgt, split=True)
                else:
                    pending[0] = lambda: epilogue(s, a2b_ps, b2a_ps, lgt)

            for s in range(NSTACK):
                stack(s)

        if reps == 1:
            body()
        else:
            # staggered_reset: no all-engine barrier on the back edge, so
            # consecutive iterations pipeline like back-to-back queued launches
            with tc.For_i(0, reps, staggered_reset=True):
                body()

    if not nc.is_finalized():
        nc.finalize()
    return nc


def shard_inputs(inputs):
    """Full inputs -> list of 8 per-core input maps."""
    A = np.asarray(inputs["A"], np.float32)
    B = np.asarray(inputs["B"], np.float32)
    maps = []
    for c in range(NCORES):
        maps.append({
            "A": np.ascontiguousarray(A[c * GSH * NA:(c + 1) * GSH * NA]),
            "B": np.ascontiguousarray(B[c * GSH * NB:(c + 1) * GSH * NB]),
            "W1": np.asarray(inputs["W1"], np.float32),
            "W2": np.asarray(inputs["W2"], np.float32),
            "bias1": np.asarray(inputs["bias1"], np.float32).reshape(1, D),
            "bias2": np.asarray(inputs["bias2"], np.float32).reshape(1, D),
            "q": np.asarray(inputs["q"], np.float32).reshape(1, NK),
        })
    return maps


_NC_CACHE = {}


def kernel(**inputs) -> np.ndarray:
    """Full (unsharded) inputs -> full [G, 2D] output, running on 8 cores."""
    from concourse.bass_utils import run_bass_kernel_spmd

    if "nc" not in _NC_CACHE:
        _NC_CACHE["nc"] = build_kernel()
    nc = _NC_CACHE["nc"]
    in_maps = shard_inputs(inputs)
    res = run_bass_kernel_spmd(nc, in_maps, core_ids=list(range(NCORES)))
    out = np.concatenate([res.results[c]["out"] for c in range(NCORES)], axis=0)
    return out.astype(np.float32)


def _ref_core(m):
    x1 = m["A"] @ m["W1"].T + m["bias1"][0]
    x2 = m["B"] @ m["W2"].T + m["bias2"][0]
    x1 = x1.reshape(GSH, H, NA, HD)
    x2 = x2.reshape(GSH, H, NB, HD)
    att = np.einsum("ghijk,k->ghij",
                    np.tanh(x1[:, :, :, None, :] * x2[:, :, None, :, :]), m["q"][0])

    def smax(v, ax):
        v = v - v.max(axis=ax, keepdims=True)
        e = np.exp(v)
        return e / e.sum(axis=ax, keepdims=True)

    b2a = smax(att.mean(axis=3), 2)
    a2b = smax(att.mean(axis=2), 2)
    A_p = np.einsum("ghik,ghi->ghk", x1, b2a).reshape(GSH, D)
    B_p = np.einsum("ghjk,ghj->ghk", x2, a2b).reshape(GSH, D)
    return np.concatenate([A_p, B_p], axis=1)


if __name__ == "__main__":
    from concourse.bass_interp import CoreSim

    reps = int(sys.argv[1]) if len(sys.argv) > 1 else 1
    trace = len(sys.argv) > 2 and sys.argv[2] == "trace"
    rng = np.random.default_rng(0)
    scale = 1.0 / np.sqrt(D)
    full = {
        "A": rng.standard_normal((G * NA, D)).astype(np.float32),
        "B": rng.standard_normal((G * NB, D)).astype(np.float32),
        "W1": (rng.standard_normal((D, D)) * scale).astype(np.float32),
        "bias1": (rng.standard_normal(D) * scale).astype(np.float32),
        "W2": (rng.standard_normal((D, D)) * scale).astype(np.float32),
        "bias2": (rng.standard_normal(D) * scale).astype(np.float32),
        "q": (rng.standard_normal(HD) * scale).astype(np.float32),
    }

    nc = build_kernel(reps=reps)
    m0 = shard_inputs(full)[0]
    sim = CoreSim(nc, trace=trace)
    for k, v in m0.items():
        sim.tensor(k)[:] = v
    sim.simulate()
    got = sim.tensor("out").copy()
    want = _ref_core(m0)
    err = np.abs(got - want).max() / np.abs(want).max()
    print("sim time:", sim.time, "ns", f"({reps} reps)")
    print("rel err:", err)
